# revision 38
# baseline (speedup 1.0000x reference)
"""Multi-head cosine self-attention on 8 Trainium2 NeuronCores (Bass/Tile).

Problem: y = MHA(x) with L2-normalized q/k (cosine attention) and per-head
scaling sim / n**sigmoid(m);  x: [4, 2048, 1024], 16 heads of dim 64.

Sharding: core c handles batch c//2 and head-group c%2 (8 heads = 512 of the
1024 q/k/v features).  Each core computes its partial output
(attn_out_part @ Wo[rows]); the host sums the two partials per batch and adds
bo.  No collectives.

Per-core pipeline (bf16 datapath, fp32 PSUM accumulation):
  - xT blocks stream to SBUF bf16; v = x Wv + bv (x-tile stationary)
  - qT/kT = W.T @ xT, k-outer loop with 2-chunk PSUM accumulators so each
    stationary weight tile is reused across i-chunks
  - row norms: ones-block matmul -> [2, n] PSUM; 1/(||q||*n^sig) computed in
    ONE Abs_reciprocal_sqrt activation (scale=(n^sig)^2); broadcast to 128
    partitions via a K=2 indicator matmul and applied in-place (DVE)
  - all norms for all 4 head-pairs are emitted before attention, so the PE
    stream stays dense and the HAM clock stays at 2.4 GHz
  - simT[j,i] = khatT.T @ qhatT per head, K=64 row-packing (2 heads
    concurrent in PE rows 0-63 / 64-127); evicted to bf16 `at` alternating
    DVE/ACT; out2T accumulated with M=64 col-packing
  - final projection aoT.T @ Wo with kt-outer loop (stationary reuse)
"""

import os
import sys

for _p in ("/opt/trn_rl_repo",):
    if os.path.isdir(_p) and _p not in sys.path:
        sys.path.insert(0, _p)

from contextlib import ExitStack

import ml_dtypes
import numpy as np

import concourse.bacc as bacc
import concourse.mybir as mybir
import concourse.tile as tile
from concourse import bass_utils

P = 128
F = 1024  # model dim
H = 16  # total heads
HD = 64  # head dim
G = 2  # head groups (tensor-parallel factor)
FG = F // G  # 512 features per core
PAIRS = FG // P  # 4 head-pairs per core
KT = F // P  # 8 contraction tiles for the projections
NCORES = 8
F32 = mybir.dt.float32
BF = mybir.dt.bfloat16
AF = mybir.ActivationFunctionType


def _mm(nc, out, lhsT, rhs, **kw):
    return nc.tensor.matmul(out, lhsT, rhs, **kw)


def build_core_program(nc, n=2048):
    NC = n // 512  # i-chunks
    NT = n // P  # n-tiles (= j-tiles)
    NTC = 512 // P  # n-tiles per i-chunk

    xt = nc.dram_tensor("xt", [P, NC, KT, 512], BF, kind="ExternalInput").ap()
    wq = nc.dram_tensor("wq", [P, PAIRS, KT, P], BF, kind="ExternalInput").ap()
    wk = nc.dram_tensor("wk", [P, PAIRS, KT, P], BF, kind="ExternalInput").ap()
    wv = nc.dram_tensor("wv", [P, KT, FG], BF, kind="ExternalInput").ap()
    wo = nc.dram_tensor("wo", [P, PAIRS, F], BF, kind="ExternalInput").ap()
    bqd = nc.dram_tensor("bq", [P, PAIRS], F32, kind="ExternalInput").ap()
    bkd = nc.dram_tensor("bk", [P, PAIRS], F32, kind="ExternalInput").ap()
    bvd = nc.dram_tensor("bv", [FG], BF, kind="ExternalInput").ap()
    # cmsq[a, p] = (n ** sigmoid(m))**2 for local head 2p+a
    cmsq = nc.dram_tensor("cmsq", [2, PAIRS], F32, kind="ExternalInput").ap()
    cind = nc.dram_tensor("cind", [2, P], BF, kind="ExternalInput").ap()
    cblk = nc.dram_tensor("cblk", [P, 2], BF, kind="ExternalInput").ap()
    cones = nc.dram_tensor("cones", [1, P], BF, kind="ExternalInput").ap()
    out = nc.dram_tensor("out", [n, F], BF, kind="ExternalOutput").ap()

    with tile.TileContext(nc) as tc, ExitStack() as ctx:
        const = ctx.enter_context(tc.tile_pool(name="const", bufs=1))
        persist = ctx.enter_context(tc.tile_pool(name="persist", bufs=1))
        ps = ctx.enter_context(tc.tile_pool(name="ps", bufs=1, space="PSUM"))
        work = ctx.enter_context(tc.tile_pool(name="work", bufs=1))

        # --- constants (issued on the ACT DGE queue so the tiny transfers
        # don't delay xall[0]/wv on the sync queue) ----------------------
        ones_blk = const.tile([P, 2], BF)  # block col-sums for head-pair norms
        nc.scalar.dma_start(ones_blk[:], cblk)
        ind = const.tile([2, P], BF)  # partition-broadcast indicator
        nc.scalar.dma_start(ind[:], cind)
        ones_row = const.tile([1, P], BF)  # bias outer-product row
        nc.scalar.dma_start(ones_row[:], cones)
        zcol = const.tile([P, 1], F32)  # explicit zero bias for ACT
        nc.any.memset(zcol[:], 0.0)

        bq_sb = const.tile([P, PAIRS], F32)
        nc.scalar.dma_start(bq_sb[:], bqd)
        bk_sb = const.tile([P, PAIRS], F32)
        nc.scalar.dma_start(bk_sb[:], bkd)
        bv_sb = const.tile([1, FG], BF)
        nc.scalar.dma_start(bv_sb[:], bvd[None, :])
        cm_sb = const.tile([2, PAIRS], F32)
        nc.scalar.dma_start(cm_sb[:], cmsq)

        # --- persistent activations -------------------------------------
        # DMA order matters: v-proj needs xall[0] + wv first; wq/wk next
        # (qk-proj starts ~40us in); wo last (needed only at the end).
        xall = persist.tile([P, NC, KT, 512], BF)
        wv_sb = persist.tile([P, KT, FG], BF)
        qT = persist.tile([P, PAIRS, n], BF)  # (x Wq + bq)^T, 2 heads/tile
        kT = persist.tile([P, PAIRS, n], BF)
        v = persist.tile([P, NT, FG], BF)  # x Wv + bv, natural layout
        aoT = persist.tile([P, PAIRS, n], BF)  # attn-out^T
        wq_sb = persist.tile([P, PAIRS, KT, P], BF)
        wk_sb = persist.tile([P, PAIRS, KT, P], BF)
        wo_sb = persist.tile([P, PAIRS, F], BF)
        # split the startup-critical 2 MB across both DGE queues; xall[1]
        # rides the scalar queue behind wv so v-proj(ic=1) isn't stuck
        # behind xall[0] on the sync queue
        nc.sync.dma_start(xall[:, 0, :4], xt[:, 0, :4])
        nc.sync.dma_start(xall[:, 0, 4:], xt[:, 0, 4:])
        nc.scalar.dma_start(wv_sb[:], wv)  # concurrent with xall[0]
        nc.scalar.dma_start(xall[:, 1], xt[:, 1])
        for ic in range(2, NC):
            nc.sync.dma_start(xall[:, ic], xt[:, ic])
        nc.scalar.dma_start(wq_sb[:], wq)
        nc.scalar.dma_start(wk_sb[:], wk)
        nc.scalar.dma_start(wo_sb[:], wo)

        # --- PE clock warm-up: ~36 junk matmuls while the DMAs land -----
        # HAM un-throttles after ~3.4us of sustained PE activity; these run
        # during the otherwise-idle startup so the first real matmul (and
        # everything after) executes at 2.4 GHz instead of 1.2 GHz.
        dum_w = const.tile([P, P], BF)
        nc.vector.memset(dum_w[:], 0.0)
        dum_x = const.tile([P, 512], BF)
        nc.vector.memset(dum_x[:], 0.0)
        for _wu in range(36):
            dps = ps.tile([P, 512], F32, tag="av0", bufs=1, name="nps")
            _mm(nc, dps, dum_w[:], dum_x[:], start=True, stop=True)

        # ================= phase 1: v projection =========================
        for ic in range(NC):
            for jt in range(NTC):
                nt_idx = ic * NTC + jt
                jsl = slice(jt * P, (jt + 1) * P)
                pt = ps.tile([P, FG], F32, tag=f"big{nt_idx % 3}", bufs=1,
                             name=f"vacc{nt_idx % 3}")
                for k in range(KT):
                    _mm(nc, pt, xall[:, ic, k, jsl], wv_sb[:, k, :],
                        start=(k == 0), stop=False)
                # + 1s^T bv outer product adds the bias to every row
                _mm(nc, pt, ones_row, bv_sb, start=False, stop=True)
                nc.scalar.activation(v[:, nt_idx, :], pt, AF.Identity,
                                     bias=zcol[:])

        # ========== phase 2: q/k projections + all norms =================
        # Norm chains interleave with the projections per (ft, q/k) so the
        # ACT-bound chain latency hides under the projection MM stream and
        # the PE never idles long enough for the HAM clock to re-throttle.
        for ft in range(PAIRS):
            for wsb, bsb, dstT, scale_ap in (
                    (wq_sb, bq_sb, qT, cm_sb[:, ft:ft + 1]),
                    (wk_sb, bk_sb, kT, None)):
                for ich in range(NC // 2):
                    ics = (2 * ich, 2 * ich + 1)
                    pts = [ps.tile([P, 512], F32, tag=f"big{i}", bufs=1,
                                   name=f"qkacc{i}")
                           for i in range(2)]
                    for k in range(KT):
                        for i, ic in enumerate(ics):
                            _mm(nc, pts[i], wsb[:, ft, k, :],
                                xall[:, ic, k, :],
                                start=(k == 0), stop=(k == KT - 1),
                                skip_group_check=True)
                    for i, ic in enumerate(ics):
                        isl = slice(ic * 512, (ic + 1) * 512)
                        nc.scalar.activation(dstT[:, ft, isl], pts[i],
                                             AF.Identity,
                                             bias=bsb[:, ft:ft + 1])
                # --- norm chain: 1/(||t|| * s) as [2, n] bf16 row ---------
                # Square on DVE (tensor_tensor mult) to keep ACT free for
                # the rsqrt + eviction work
                sq = work.tile([P, n], BF, tag="sq", bufs=2)
                nc.vector.tensor_tensor(sq[:], dstT[:, ft, :], dstT[:, ft, :],
                                        mybir.AluOpType.mult)
                rowr = work.tile([2, n], BF, tag="rowr", bufs=2)
                for ch in range(NC):
                    csl = slice(ch * 512, (ch + 1) * 512)
                    nps = ps.tile([2, 512], F32, tag="av0", bufs=1, name="nps")
                    _mm(nc, nps, ones_blk, sq[:, csl], start=True, stop=True)
                    if scale_ap is not None:
                        nc.scalar.activation(rowr[:, csl], nps,
                                             AF.Abs_reciprocal_sqrt,
                                             bias=zcol[:2], scale=scale_ap)
                    else:
                        nc.scalar.activation(rowr[:, csl], nps,
                                             AF.Abs_reciprocal_sqrt,
                                             bias=zcol[:2])
                # broadcast row across partitions and apply in place
                for ch in range(NC):
                    csl = slice(ch * 512, (ch + 1) * 512)
                    bps = ps.tile([P, 512], F32, tag="av1", bufs=1, name="bps")
                    _mm(nc, bps, ind, rowr[:, csl], start=True, stop=True)
                    nc.vector.tensor_tensor(dstT[:, ft, csl],
                                            dstT[:, ft, csl],
                                            bps, mybir.AluOpType.mult)

        # ========== phase 3: cosine attention + output projection ========
        # Software-pipelined j-loop (lag 2): the PE queue is strict FIFO, so
        # av(j) — which waits on the cross-engine eviction of at(j) — must
        # sit BEHIND sim(j+1)/sim(j+2) in the queue or it head-of-line
        # blocks ready sims and the HAM clock re-throttles.
        # i-chunk-outer / pair-inner ordering so each chunk's output
        # projection (and its out DMA) interleaves with the next chunk's
        # attention instead of draining in a tail phase.
        LAG = 3
        for ic in range(NC):
            isl = slice(ic * 512, (ic + 1) * 512)
            for pr in range(PAIRS):
                avp = ps.tile([P, 512], F32, tag=f"av{pr % 2}", bufs=1,
                              name=f"av{pr % 2}")
                ats = {}

                def emit_sim(j):
                    jsl = slice(j * P, (j + 1) * P)
                    sp2 = ps.tile([P, 1024], F32, tag=f"big{j % 3}", bufs=1,
                                  name=f"sp{j % 3}")
                    for po in (0, HD):  # head 2pr (rows 0-63), 2pr+1
                        _mm(nc, sp2[:, 8 * po:8 * po + 512],
                            kT[po:po + HD, pr, jsl],
                            qT[po:po + HD, pr, isl],
                            start=True, stop=True, tile_position=(po, 0))
                    at = work.tile([P, 1024], BF, tag="at", bufs=6, name="at")
                    # whole-tile eviction on alternating engines: one sem for
                    # both av halves (keeps the av pair concurrent) and less
                    # per-op overhead; the lag-2 pipeline covers the latency
                    if j % 2 == 0:
                        nc.scalar.copy(at[:], sp2)
                    else:
                        nc.vector.tensor_copy(at[:], sp2)
                    ats[j] = at

                # j-paired emission [sim(j+2), sim(j+3), av(j), av(j+1)]:
                # back-to-back sim pairs hide each other's LDWEIGHTS via
                # row-group alternation; only the av weight loads stay
                # exposed.  PSUM-safe: a sim slot frees on eviction (not on
                # av consumption), so 3 slots still suffice.
                for j in range(LAG):
                    emit_sim(j)
                for jb in range(0, NT, 2):
                    for jj in (jb + LAG, jb + LAG + 1):
                        if jj < NT:
                            emit_sim(jj)
                    for j in (jb, jb + 1):
                        at = ats.pop(j)
                        for po in (0, HD):
                            _mm(nc, avp[po:po + HD, :],
                                v[:, j, pr * P + po:pr * P + po + HD],
                                at[:, 8 * po:8 * po + 512],
                                start=(j == 0), stop=(j == NT - 1),
                                tile_position=(0, po), skip_group_check=True)
                if pr % 2 == 0:
                    nc.vector.tensor_copy(aoT[:, pr, isl], avp)
                else:
                    nc.scalar.copy(aoT[:, pr, isl], avp)

            # --- output projection for this i-chunk ----------------------
            # kt-outer: the aoT stationary is reused by the second fc MM
            # (ldweights=True), both fc accumulators live simultaneously
            for jt in range(NTC):
                nt = ic * NTC + jt
                ntsl = slice(nt * P, (nt + 1) * P)
                ost = work.tile([P, F], BF, tag="ost", bufs=2)
                pt2s = [ps.tile([P, 512], F32, tag=f"av{fc}", bufs=1,
                                name=f"av{fc}")
                        for fc in range(2)]
                for kt in range(PAIRS):
                    for fc in range(2):
                        fsl = slice(fc * 512, (fc + 1) * 512)
                        _mm(nc, pt2s[fc], aoT[:, kt, ntsl],
                            wo_sb[:, kt, fsl],
                            start=(kt == 0), stop=(kt == PAIRS - 1),
                            skip_group_check=True)
                for fc in range(2):
                    fsl = slice(fc * 512, (fc + 1) * 512)
                    if fc % 2 == 0:
                        nc.vector.tensor_copy(ost[:, fsl], pt2s[fc])
                    else:
                        nc.scalar.copy(ost[:, fsl], pt2s[fc])
                nc.sync.dma_start(out[ntsl, :], ost[:])
    return nc


_CACHE = {}


def get_nc(n=2048):
    if n not in _CACHE:
        nc = bacc.Bacc("TRN2", target_bir_lowering=False, debug=False,
                       num_devices=NCORES)
        build_core_program(nc, n)
        nc.compile()
        _CACHE[n] = nc
    return _CACHE[n]


def _bf(a):
    return np.ascontiguousarray(a).astype(ml_dtypes.bfloat16)


def _warr(W, sl):
    return _bf(
        np.asarray(W, np.float32)[:, sl].reshape(KT, P, FG).transpose(1, 0, 2))


def _warr_ft(W, sl):
    return _bf(
        np.asarray(W, np.float32)[:, sl].reshape(KT, P, PAIRS, P)
        .transpose(1, 2, 0, 3))


_IND = np.zeros((2, P), np.float32)
_IND[0, :HD] = 1.0
_IND[1, HD:] = 1.0
_BLK = np.zeros((P, 2), np.float32)
_BLK[:HD, 0] = 1.0
_BLK[HD:, 1] = 1.0
_ONES = np.ones((1, P), np.float32)


def make_in_maps(x, Wq, bq, Wk, bk, Wv, bv, Wo, bo, m):
    n = x.shape[1]
    sig = 1.0 / (1.0 + np.exp(-np.asarray(m, np.float64)))
    scale = np.float64(n) ** sig  # [16] per-head n^sigmoid(m)
    NCc = n // 512
    # xt is shared by the two cores of each batch; weight transforms are
    # shared by the four cores of each head-group — build each variant once
    xts = [
        _bf(np.asarray(x[bi], np.float32)
            .reshape(NCc, 512, KT, P).transpose(3, 0, 2, 1))
        for bi in range(x.shape[0])
    ]
    gmaps = []
    for g in range(G):
        sl = slice(g * FG, (g + 1) * FG)
        hsc = scale[g * (H // G):(g + 1) * (H // G)]  # 8 local heads
        cm = (hsc ** 2).reshape(PAIRS, 2).T  # [2, PAIRS]
        gmaps.append({
            "wq": _warr_ft(Wq, sl), "wk": _warr_ft(Wk, sl), "wv": _warr(Wv, sl),
            "wo": _bf(
                np.asarray(Wo, np.float32)[sl].reshape(PAIRS, P, F)
                .transpose(1, 0, 2)),
            "bq": np.ascontiguousarray(np.asarray(bq, np.float32)[sl].reshape(PAIRS, P).T),
            "bk": np.ascontiguousarray(np.asarray(bk, np.float32)[sl].reshape(PAIRS, P).T),
            "bv": _bf(np.asarray(bv, np.float32)[sl]),
            "cmsq": np.ascontiguousarray(cm.astype(np.float32)),
            "cind": _bf(_IND),
            "cblk": _bf(_BLK),
            "cones": _bf(_ONES),
        })
    return [{"xt": xts[c // 2], **gmaps[c % 2]} for c in range(NCORES)]


def kernel(x, Wq, bq, Wk, bk, Wv, bv, Wo, bo, m, _trace=False):
    x = np.asarray(x, np.float32)
    b, n, f = x.shape
    nc = get_nc(n)
    in_maps = make_in_maps(x, Wq, bq, Wk, bk, Wv, bv, Wo, bo, m)
    res = bass_utils.run_bass_kernel_spmd(nc, in_maps,
                                          core_ids=list(range(NCORES)),
                                          trace=_trace)
    outs = [r["out"] for r in res.results]
    y = np.empty((b, n, f), np.float32)
    for bi in range(b):
        y[bi] = (outs[2 * bi].astype(np.float32)
                 + outs[2 * bi + 1].astype(np.float32))
    y += np.asarray(bo, np.float32).reshape(1, 1, f)
    if _trace:
        kernel._last_results = res
    return y


if __name__ == "__main__":
    # build-only smoke test (no device)
    nc = bacc.Bacc("TRN2", target_bir_lowering=False, debug=False,
                   num_devices=NCORES)
    build_core_program(nc, n=int(sys.argv[1]) if len(sys.argv) > 1 else 2048)
    print("build OK")
